# revision 26
# baseline (speedup 1.0000x reference)
"""Trainium2 Bass kernel for nn_Attention (pooling attention).

Math (per batch b):
    u[b]     = W_score @ h_t[b]            (score = (hidden @ W_score) . h_t
                                            collapses to hidden . (W_score @ h_t))
    score[t] = hidden[b,t,:] . u[b]        (DVE fused tensor_tensor_reduce:
                                            f16 mul + fp32 add-reduce per t-tile)
    p[t]     = exp(score[t] - 50)          (ScalarE, bf16 out, fused accum -> q)
    ctx      = sum_t p[t] * hidden[b,t,:]  (PE bf16xf16 matmuls, UNNORMALIZED,
                                            accumulated into ONE [16, 256] PSUM
                                            tile via zero-padded p columns: row b
                                            of the PSUM tile = batch b's ctx, so
                                            no per-batch scatter/transpose)
    out[b]   = tanh([ctx/s, h_t[b]] @ W_att)

Design notes vs the previous version (120us):
  - score moved off the mul+tree+reduce DVE chain (2.6us/half-batch) onto
    tensor_tensor_reduce: one instr per 128x256 t-tile, f16 2x packed mul with
    fp32 accumulator (~194ns).  Kills both the DVE bottleneck and the 1.9us/batch
    ACT broadcast-materialization of u (TTR's in1 is a plain [P, 256] tile).
  - exp writes p into a zero-padded [P, 16i, 16col] per-batch strip (col b), so
    the ctx matmuls use lhsT = [128, 16] padded columns and all 256 of them
    accumulate into a single persistent [16, 256] PSUM tile.  Rows b' != b get
    += 0.  This deletes the 32 fp32 scatter matmuls and the per-batch PSUM->SBUF
    ctx copies.
  - softmax normalization deferred to one tensor_scalar at the end
    (ctx * (1/s) with per-partition scalar), then two 16x128 transpose matmuls
    feed the final W_att matmul.
  - DMA flood at 1MB (full batch) per transfer, alternating the two HWDGE rings
    (sync/scalar) so ring-FIFO bubbles overlap.

hidden_states is staged to HBM as fp16 host-side; h_t rides along fp32.
Sharding: data-parallel over batch, 16 batches per core on 8 cores; weights
replicated.
"""

import sys

import numpy as np

_TRN_REPO = "/opt/trn_rl_repo"
if _TRN_REPO not in sys.path:
    sys.path.insert(0, _TRN_REPO)

import concourse.bass as bass
import concourse.bacc as bacc
import concourse.tile as tile
from concourse import mybir
from concourse.bass_utils import run_bass_kernel_spmd

N_CORES = 8
B, T, H = 128, 2048, 256
NB = B // N_CORES  # batches per core
P = 128  # SBUF partitions
TT = T // P  # t-tiles per batch (16)
NDVE = 11  # t-tiles per batch scored on DVE (STT); the rest go to the PE
NPE = TT - NDVE  # t-tiles scored on PE from transposed-staged chunks
OUT_D = 128
EXP_SHIFT = -50.0  # keeps exp() in fp32/bf16 range; cancels in the softmax ratio

F32 = mybir.dt.float32
F16 = mybir.dt.float16
BF16 = mybir.dt.bfloat16


def _build_kernel(nc: bass.Bass, tc: "tile.TileContext", hiddenc, ht32, wst, watt, ident, out):
    mult = mybir.AluOpType.mult
    add = mybir.AluOpType.add

    from contextlib import ExitStack

    with ExitStack() as ctx:
        const = ctx.enter_context(tc.tile_pool(name="const", bufs=1))
        ybufs = ctx.enter_context(tc.tile_pool(name="ybufs", bufs=5))
        sc = ctx.enter_context(tc.tile_pool(name="sc", bufs=2))
        psum_t = ctx.enter_context(tc.tile_pool(name="psum_t", bufs=3, space="PSUM"))
        psum_u = ctx.enter_context(tc.tile_pool(name="psum_u", bufs=2, space="PSUM"))
        psum_p = ctx.enter_context(tc.tile_pool(name="psum_p", bufs=1, space="PSUM"))

        # ---- setup loads: all via the gpsimd SWDGE ring, keeping both HWDGE
        # rings (sync/scalar) free for the y flood from the first cycle.
        ht_sb = const.tile([NB, H], F32, tag="ht")
        nc.gpsimd.dma_start(out=ht_sb, in_=ht32[:, :])
        ident_sb = const.tile([16, 16], F32, tag="ident")
        nc.gpsimd.dma_start(out=ident_sb, in_=ident[:, :])
        ident16f = const.tile([16, 16], F16, tag="ident16f")
        wst_sb = const.tile([P, 2, H], F16, tag="wst")  # W_score^T as [k, kk, h]
        nc.gpsimd.dma_start(out=wst_sb, in_=wst.rearrange("p (kk h) -> p kk h", kk=2))
        watt_sb = const.tile([P, 4, OUT_D], F16, tag="watt")  # W_att as [d, dd, j]
        nc.gpsimd.dma_start(out=watt_sb, in_=watt.rearrange("p (dd j) -> p dd j", dd=4))

        # zero-padded p storage: per batch a [16i, 16col] strip; only col b is
        # ever written (by exp), so ctx matmuls see 0 for every other row of
        # the shared [16, 256] ctx accumulator.  Emitted first so the memset
        # runs during the engine preamble / DMA warmup.
        p_pad = const.tile([P, NB, TT, NB], BF16, tag="p_pad")
        nc.vector.memset(p_pad.rearrange("p b i c -> p (b i c)"), 0.0)

        ones_col = const.tile([P, 1], F32, tag="ones_col")
        nc.vector.memset(ones_col, 1.0)
        shift_col = const.tile([P, 1], F32, tag="shift_col")
        nc.vector.memset(shift_col, EXP_SHIFT)

        q_all = const.tile([P, NB], F32, tag="q_all")

        # ---- y-load flood: one combined 1.36MB transfer per batch carrying
        # both the [t,h] tile block and the transposed score chunks,
        # alternating the two HWDGE rings (sync/scalar).
        YC = TT * H + 2 * NPE * P
        ylist = {}

        def emit_load(b):
            eng = nc.sync if b % 2 == 0 else nc.scalar
            yc = ybufs.tile([P, YC], F16, tag="yc", name=f"yc_{b}")
            eng.dma_start(out=yc, in_=hiddenc[b])
            ylist[b] = yc

        PREF = 4  # batches of DMA-ahead
        for k in range(PREF):
            emit_load(k)

        nc.scalar.copy(out=ident16f, in_=ident_sb)

        # ---- h_t^T and u = h_t @ W_score^T (full fp32) ---------------------
        htT_sb = const.tile([P, 2, NB], F16, tag="htT")  # h_t^T halves [k, half, b]
        for half in range(2):
            ps_tr = psum_t.tile([P, NB], F32, tag="ptmp", name=f"ps_tr{half}")
            nc.tensor.matmul(
                ps_tr,
                lhsT=ht_sb[:, half * P : (half + 1) * P],
                rhs=ident_sb,
                start=True,
                stop=True,
            )
            nc.scalar.copy(out=htT_sb[:, half, :], in_=ps_tr)

        ps_u = psum_t.tile([NB, H], F32, tag="ptmp")
        for half in range(2):
            nc.tensor.matmul(
                ps_u,
                lhsT=htT_sb[:, half, :],
                rhs=wst_sb[:, half, :],
                start=(half == 0),
                stop=(half == 1),
            )
        u16_sb = const.tile([NB, H], F16, tag="u16")
        nc.scalar.copy(out=u16_sb, in_=ps_u)
        # u^T halves [h-part, half, b] for the PE score tiles
        uT_sb = const.tile([P, 2, NB], F16, tag="uT")
        for half in range(2):
            ps_ut = psum_t.tile([P, NB], F32, tag="ptmp", name=f"ps_ut{half}")
            nc.tensor.matmul(
                ps_ut,
                lhsT=u16_sb[:, half * P : (half + 1) * P],
                rhs=ident16f,
                start=True,
                stop=True,
            )
            nc.scalar.copy(out=uT_sb[:, half, :], in_=ps_ut)

        # ---- persistent PSUM accumulators ----------------------------------
        # ctx for ALL batches: row b = unnormalized ctx of batch b.
        ctx_ps = psum_p.tile([NB, H], F32, tag="ctx_all", name="ctx_all")
        # final output accumulator: do the h_t @ W_att half at setup time
        out_ps = psum_p.tile([NB, OUT_D], F32, tag="out_ps", name="out_ps")
        for dd in range(2, 4):
            nc.tensor.matmul(
                out_ps,
                lhsT=htT_sb[:, dd - 2, :],
                rhs=watt_sb[:, dd, :],
                start=(dd == 2),
                stop=False,
            )

        # u[b] broadcast to all partitions for batch 0 (PE ones-matmul), then
        # a small ACT copy to a plain f16 tile for the TTR in1.
        def emit_ubc(b):
            ubc_ps = psum_u.tile([P, H], F32, tag="ubc", name=f"ubc{b}")
            sel = ident16f[:, b : b + 1].broadcast_to([16, P])
            nc.tensor.matmul(ubc_ps, lhsT=sel, rhs=u16_sb, start=True, stop=True)
            ubc16 = sc.tile([P, H], F16, tag="ubc16", name=f"ubc16_{b}")
            nc.scalar.copy(out=ubc16, in_=ubc_ps)
            return ubc16

        ubc_next = emit_ubc(0)

        # ---- per-batch pipeline --------------------------------------------
        for b in range(NB):
            yc = ylist.pop(b)
            y16 = yc[:, 0 : TT * H].rearrange("p (i h) -> p i h", h=H)
            yt16 = yc[:, TT * H :].rearrange("p (c t) -> p c t", t=P)
            ubc16 = ubc_next

            # score for tiles [0, NDVE): fused DVE mul+reduce (fp32 accum):
            # scalar_tensor_tensor, out = (y*1)*u, accum = sum(out).
            score32 = sc.tile([P, TT], F32, tag="score32", name=f"s32_{b}")
            for i in range(NDVE):
                z = sc.tile([P, H], F16, tag="z")
                nc.vector.scalar_tensor_tensor(
                    out=z,
                    in0=y16[:, i, :],
                    scalar=1.0,
                    in1=ubc16,
                    op0=mult,
                    op1=mult,
                    accum_out=score32[:, i : i + 1],
                )

            # score for tiles [NDVE, TT): PE stationary-weight matmuls over
            # the transposed chunks (yT^T @ u_half accumulated over h-halves
            # gives the [128t, 1] score column directly).
            scoreT_ps = psum_t.tile([P, NPE], F32, tag="ptmp", name=f"sT_{b}")
            for j in range(NPE):
                for half in range(2):
                    nc.tensor.matmul(
                        scoreT_ps[:, j : j + 1],
                        lhsT=yt16[:, 2 * j + half, :],
                        rhs=uT_sb[:, half, b : b + 1],
                        start=(half == 0),
                        stop=(half == 1),
                    )
            nc.scalar.copy(out=score32[:, NDVE:TT], in_=scoreT_ps)

            # next batch's u broadcast rides the PE queue ahead of the ctx
            # matmuls so the DVE never waits on it.
            if b + 1 < NB:
                ubc_next = emit_ubc(b + 1)
            if b + PREF < NB:
                emit_load(b + PREF)

            # p = exp(score - 50) in bf16 into the padded column strip;
            # per-batch sum rides the ACT accumulator.
            nc.scalar.activation(
                out=p_pad[:, b, :, b],
                in_=score32,
                func=mybir.ActivationFunctionType.Exp,
                bias=shift_col,
                scale=1.0,
                accum_out=q_all[:, b : b + 1],
            )

            # ctx[row b] += sum_i p[:, i] * y[:, i, :]  (unnormalized)
            for i in range(TT):
                nc.tensor.matmul(
                    ctx_ps,
                    lhsT=p_pad[:, b, i, :],
                    rhs=y16[:, i, :],
                    start=(b == 0 and i == 0),
                    stop=(b == NB - 1 and i == TT - 1),
                )

        # ---- finalize: s per batch, normalize, transpose, @ W_att, tanh ----
        s_ps = psum_t.tile([NB, 1], F32, tag="ptmp", name="s_ps")
        nc.tensor.matmul(s_ps, lhsT=q_all, rhs=ones_col, start=True, stop=True)
        s_sb = sc.tile([NB, 1], F32, tag="s_sb")
        nc.scalar.copy(out=s_sb, in_=s_ps)
        rs_sb = sc.tile([NB, 1], F32, tag="rs_sb")
        nc.vector.reciprocal(out=rs_sb, in_=s_sb)

        preN = sc.tile([NB, H], F16, tag="preN")
        nc.vector.tensor_scalar_mul(preN, ctx_ps, rs_sb)

        preT = sc.tile([P, 2, NB], F16, tag="preT")
        for j in range(2):
            pT_ps = psum_t.tile([P, NB], F32, tag="ptmp", name=f"pT{j}")
            nc.tensor.matmul(
                pT_ps,
                lhsT=preN[:, j * P : (j + 1) * P],
                rhs=ident16f,
                start=True,
                stop=True,
            )
            nc.scalar.copy(out=preT[:, j, :], in_=pT_ps)
        for dd in range(2):
            nc.tensor.matmul(
                out_ps,
                lhsT=preT[:, dd, :],
                rhs=watt_sb[:, dd, :],
                start=False,
                stop=(dd == 1),
            )
        out_sb = sc.tile([NB, OUT_D], F32, tag="out_sb")
        nc.scalar.activation(
            out=out_sb, in_=out_ps, func=mybir.ActivationFunctionType.Tanh
        )
        nc.sync.dma_start(out=out[:, :], in_=out_sb)


_NC_CACHE = {}


def _get_nc():
    if "nc" not in _NC_CACHE:
        nc = bacc.Bacc("TRN2", target_bir_lowering=False, debug=False)
        hiddenc = nc.declare_dram_parameter(
            "hiddenc", [NB, P, TT * H + 2 * NPE * P], F16, isOutput=False
        )
        ht32 = nc.declare_dram_parameter("ht32", [NB, H], F32, isOutput=False)
        wst = nc.declare_dram_parameter("w_score_t", [P, 2 * H], F16, isOutput=False)
        watt = nc.declare_dram_parameter("w_att", [P, 4 * OUT_D], F16, isOutput=False)
        ident = nc.declare_dram_parameter("ident16", [16, 16], F32, isOutput=False)
        out = nc.declare_dram_parameter("out", [NB, OUT_D], F32, isOutput=True)
        with tile.TileContext(nc) as tc:
            _build_kernel(nc, tc, hiddenc, ht32, wst, watt, ident, out)
        nc.compile()
        _NC_CACHE["nc"] = nc
    return _NC_CACHE["nc"]


def _run(hidden_states, W_score, W_att, trace=False, trace_kwargs=None):
    hidden_states = np.asarray(hidden_states, dtype=np.float32)
    W_score = np.asarray(W_score, dtype=np.float32)
    W_att = np.ascontiguousarray(
        np.asarray(W_att, dtype=np.float16).reshape(4, P, OUT_D).transpose(1, 0, 2).reshape(P, 4 * OUT_D)
    )
    hidden16 = hidden_states.astype(np.float16)
    # combined per-batch region: [t,h] tile block (t = p*TT + i) followed by
    # the transposed chunks for the PE-scored tiles (partition = h there).
    hv = hidden16.reshape(B, P, TT, 2, P)  # [b, p, i, half, h]
    hiddenc = np.concatenate(
        [
            hidden16.reshape(B, P, TT * H),
            hv[:, :, NDVE:, :, :].transpose(0, 4, 2, 3, 1).reshape(B, P, 2 * NPE * P),
        ],
        axis=2,
    )
    ht32 = np.ascontiguousarray(hidden_states[:, T - 1, :])
    wst = np.ascontiguousarray(
        W_score.T.astype(np.float16).reshape(2, P, H).transpose(1, 0, 2).reshape(P, 2 * H)
    )
    ident = np.eye(16, dtype=np.float32)

    nc = _get_nc()
    in_maps = []
    for c in range(N_CORES):
        in_maps.append(
            {
                "hiddenc": hiddenc[c * NB : (c + 1) * NB],
                "ht32": ht32[c * NB : (c + 1) * NB],
                "w_score_t": wst,
                "w_att": W_att,
                "ident16": ident,
            }
        )
    kwargs = {}
    if trace:
        kwargs["trace"] = True
        if trace_kwargs:
            kwargs.update(trace_kwargs)
    res = run_bass_kernel_spmd(nc, in_maps, list(range(N_CORES)), **kwargs)
    out = np.concatenate([res.results[c]["out"] for c in range(N_CORES)], axis=0)
    return out, res


def kernel(hidden_states, W_score, W_att):
    out, _ = _run(hidden_states, W_score, W_att, trace=False)
    return out
